# revision 13
# baseline (speedup 1.0000x reference)
"""Trainium2 Bass kernel for EventTransformerConv (3x TransformerConv + mean-pool + linear).

Sharding: by graph (8 graphs per core). Edges live on the core owning their dst
node. Per layer: q-side projections are folded (host) into a single affine map
W_all so that  alpha*sqrt(H) = q'.h[src] + qe.edge_attr + qc  with
q' = q@wk.T, qe = q@we.T, qc = q.(bk+be); message aggregation gathers h[src]
only and applies wv/we after the segment reduction:
   out = (S.T@[Hg|EA|1]) -> [aggH|aggEA|denom];  h' = (aggH@wv + aggEA@we)/denom
         + (bv+be) + h@ws + bs.
Scatter/softmax structure: edges grouped in 128-edge blocks, each block assigned
to an aligned 32-node window; the scatter is a PE matmul with a masked weight
matrix Sx[e, j] = (seg[e]==j) * exp(alpha[e]) built by one dual-op DVE
tensor_scalar. exp needs no max-subtraction (|alpha| < 4 for this model family;
softmax is shift-invariant so this is mathematically exact whenever exp doesn't
overflow; fp32 handles |alpha| < 80).
"""

import numpy as np
import ml_dtypes

import concourse.bacc as bacc
import concourse.tile as tile
import concourse.mybir as mybir
from concourse import bass_utils

dt = mybir.dt
FP16 = dt.float16
FP32 = dt.float32
AF = mybir.ActivationFunctionType
OP = mybir.AluOpType

NCORES = 8
H = 128
EDIM = 16
C = 8
WIN = 32            # nodes per scatter window (PSUM quadrant granularity)
KB = 128            # edges per block
GBLK = 32           # blocks per gather chunk


# ---------------------------------------------------------------- host side --

def _f16(x):
    return np.ascontiguousarray(np.asarray(x, np.float32).astype(np.float16))


def _preprocess(x, edge_index, edge_attr, batch, params):
    N, F_IN = x.shape
    E = edge_index.shape[1]
    G = int(batch.max()) + 1
    src = np.asarray(edge_index[0], np.int64)
    dst = np.asarray(edge_index[1], np.int64)
    batch = np.asarray(batch, np.int64)
    gper = G // NCORES

    # node shard boundaries (graph-aligned)
    node_start = np.searchsorted(batch, np.arange(0, G + 1, gper))
    cnts = np.diff(node_start)
    NLOC = int(-(-cnts.max() // 128) * 128)
    CH = NLOC // 128
    NWIN = NLOC // WIN
    NTOT = NCORES * NLOC
    assert NTOT < 32768

    owner = np.searchsorted(node_start, np.arange(N), side="right") - 1
    gpad = owner * NLOC + (np.arange(N) - node_start[owner])  # padded global idx

    # per (core, window) edge lists
    ecore = owner[dst]
    dst_loc = dst - node_start[ecore]
    ewin = dst_loc // WIN
    per = [[[] for _ in range(NWIN)] for _ in range(NCORES)]
    order = np.lexsort((src, dst))
    for e in order:
        per[ecore[e]][ewin[e]].append(e)
    bpw = np.zeros(NWIN, np.int64)
    for w in range(NWIN):
        mx = max(len(per[c][w]) for c in range(NCORES))
        bpw[w] = max(1, -(-mx // KB))
    NBLK = int(bpw.sum())
    phase = np.zeros(NBLK, np.int64)
    wofb = np.zeros(NBLK, np.int64)
    slot = np.zeros(NBLK, np.int64)
    pctr = [0, 0, 0, 0]
    b = 0
    for w in range(NWIN):
        for _ in range(bpw[w]):
            wofb[b] = w
            phase[b] = w % 4
            slot[b] = pctr[w % 4]
            pctr[w % 4] += 1
            b += 1
    NSLOT = max(pctr)

    x16 = _f16(x)
    ea16 = _f16(edge_attr)

    per_core = []
    for c in range(NCORES):
        src_g = np.zeros((NBLK, KB), np.int16)
        seg = np.full((NBLK, KB), -1.0, np.float32)
        Mt = np.zeros((128, NSLOT * 128), np.float16)
        ea17 = np.zeros((NBLK, KB, EDIM + 1), np.float16)
        xg = np.zeros((NBLK, KB, F_IN), np.float16)
        b = 0
        for w in range(NWIN):
            edges = per[c][w]
            for j in range(bpw[w]):
                chunk = edges[j * KB:(j + 1) * KB]
                ne = len(chunk)
                if ne:
                    ce = np.asarray(chunk, np.int64)
                    src_g[b, :ne] = gpad[src[ce]]
                    sg = (dst[ce] - node_start[c] - w * WIN).astype(np.int64)
                    seg[b, :ne] = sg.astype(np.float32)
                    ea17[b, :ne, :EDIM] = ea16[ce]
                    ea17[b, :ne, EDIM] = 1.0
                    xg[b, :ne] = x16[src[ce]]
                    Mt[phase[b] * 32 + sg, slot[b] * 128 + np.arange(ne)] = 1.0
                b += 1
        idx = src_g.reshape(NBLK * KB)
        idx16 = np.zeros((16, NBLK * KB // 16), np.int16)
        idx16[np.arange(NBLK * KB) % 16, np.arange(NBLK * KB) // 16] = idx
        idx128 = np.tile(idx16, (8, 1))

        # x feature-major for own nodes
        cnt = cnts[c]
        xf = np.zeros((F_IN, NLOC), np.float16)
        xf[:, :cnt] = x16[node_start[c]:node_start[c] + cnt].T

        # pooling matrix [NLOC, 8] -> [128, CH*8]
        gcnt = np.bincount(batch, minlength=G).astype(np.float32)
        P = np.zeros((NLOC, gper), np.float32)
        for n in range(cnt):
            g = batch[node_start[c] + n]
            P[n, g - c * gper] = 1.0 / gcnt[g]
        P = P.reshape(CH, 128, gper).transpose(1, 0, 2).reshape(128, CH * gper)

        per_core.append(dict(
            idx=idx128,
            seg=np.ascontiguousarray(seg.T),                       # [128, NBLK]
            mt=Mt,
            ea17=np.ascontiguousarray(ea17.transpose(1, 0, 2).reshape(128, NBLK * (EDIM + 1))),
            xg=np.ascontiguousarray(xg.transpose(1, 0, 2).reshape(128, NBLK * F_IN)),
            xfm=xf,
            pool=_f16(P),
        ))

    # parameter folding (fp32 on host)
    layers = []
    sH = 1.0 / np.sqrt(np.float32(H))
    for li, key in enumerate(["conv1", "conv2", "conv3"]):
        p = {k: np.asarray(v, np.float32) for k, v in params[key].items()}
        F = p["wq"].shape[0]
        Wq = p["wq"] @ p["wk"].T * sH                      # [F, F]
        We_ = p["wq"] @ p["we"].T * sH                     # [F, 16]
        Wc = (p["wq"] @ (p["bk"] + p["be"]))[:, None] * sH  # [F, 1]
        W_all = np.concatenate([Wq, We_, Wc], 1)           # [F, F+17]
        b_all = np.concatenate([p["bq"] @ p["wk"].T, p["bq"] @ p["we"].T,
                                [p["bq"] @ (p["bk"] + p["be"])]]) * sH  # [F+17]
        hbias = p["bv"] + p["be"] + p["bs"]                # [H]
        layers.append(dict(
            F=F,
            W_all=_f16(W_all),
            qqeb=np.tile(b_all.astype(np.float32), (128, 1)),
            wv=_f16(p["wv"]), we=_f16(p["we"]), ws=_f16(p["ws"]),
            hbias=np.tile(hbias.astype(np.float32), (128, 1)),
        ))

    lin_w = _f16(np.asarray(params["lin_w"], np.float32))
    lin_b = np.tile(np.asarray(params["lin_b"], np.float32), (gper, 1))

    dims = dict(N=N, E=E, G=G, F_IN=F_IN, NLOC=NLOC, CH=CH, NWIN=NWIN,
                NTOT=NTOT, NBLK=NBLK, NSLOT=NSLOT, gper=gper,
                bpw=bpw.tolist(), wofb=wofb.tolist(), phase=phase.tolist(),
                slot=slot.tolist())
    shared = dict(layers=layers, lin_w=lin_w, lin_b=lin_b)
    return dims, shared, per_core, node_start, cnts


# -------------------------------------------------------------- device side --

def _build(dims, shared):
    import os
    NLOC, CH, NWIN, NTOT = dims["NLOC"], dims["CH"], dims["NWIN"], dims["NTOT"]
    NBLK, NSLOT, F_IN = dims["NBLK"], dims["NSLOT"], dims["F_IN"]
    gper = dims["gper"]
    bpw, wofb, phase, slot = dims["bpw"], dims["wofb"], dims["phase"], dims["slot"]
    layers = shared["layers"]

    nc = bacc.Bacc("TRN2", target_bir_lowering=False, debug=False,
                   num_devices=NCORES)

    def inp(name, shape, d=FP16):
        return nc.dram_tensor(name, shape, d, kind="ExternalInput").ap()

    IDX = inp("idx", [128, NBLK * 8], dt.int16)
    SEG = inp("seg", [128, NBLK], FP32)
    MT = inp("mt", [128, NSLOT * 128])
    EA = inp("ea17", [128, NBLK * 17])
    XG = inp("xg", [128, NBLK * F_IN])
    XFM = inp("xfm", [F_IN, NLOC])
    POOL = inp("pool", [128, CH * gper])
    IDENT = inp("ident", [128, 128])
    IOTA = inp("iota32", [128, WIN])
    LINW = inp("lin_w", [H, C])
    LINB = inp("lin_b", [gper, C], FP32)
    LW, LB, LV, LE, LS, LH = [], [], [], [], [], []
    for li, L in enumerate(layers):
        F = L["F"]
        LW.append(inp(f"wall{li}", [F, F + 17]))
        LB.append(inp(f"qqeb{li}", [128, F + 17], FP32))
        LV.append(inp(f"wv{li}", [F, H]))
        LE.append(inp(f"we{li}", [EDIM, H]))
        LS.append(inp(f"ws{li}", [F, H]))
        LH.append(inp(f"hbias{li}", [128, H], FP32))
    OUT = nc.dram_tensor("out", [gper, C], FP32, kind="ExternalOutput").ap()
    HEXT = [nc.dram_tensor("hext0", [NTOT, H], FP16, kind="ExternalOutput").ap(),
            nc.dram_tensor("hext1", [NTOT, H], FP16, kind="ExternalOutput").ap()]

    NCHUNK = -(-NBLK // GBLK)

    with tile.TileContext(nc) as tc:
        with tc.tile_pool(name="res", bufs=1) as res, \
             tc.tile_pool(name="gat", bufs=3) as gat, \
             tc.tile_pool(name="xs", bufs=3) as xs, \
             tc.tile_pool(name="blk", bufs=4) as blk, \
             tc.tile_pool(name="chk", bufs=3) as chk, \
             tc.tile_pool(name="sbp", bufs=3) as sbp, \
             tc.tile_pool(name="psq", bufs=2, space="PSUM") as psq, \
             tc.tile_pool(name="pso", bufs=2, space="PSUM") as pso, \
             tc.tile_pool(name="psp", bufs=1, space="PSUM") as psp, \
             tc.tile_pool(name="dram", bufs=1, space="DRAM") as dram:

            # resident loads
            idx_t = res.tile([128, NBLK * 8], dt.int16)
            seg_t = res.tile([128, NBLK], FP32)
            mt_t = res.tile([128, NSLOT * 128], FP16)
            ea_t = res.tile([128, NBLK * 17], FP16)
            pool_t = res.tile([128, CH * gper], FP16)
            id_t = res.tile([128, 128], FP16)
            io_t = res.tile([128, WIN], FP16)
            linw_t = res.tile([H, C], FP16)
            linb_t = res.tile([gper, C], FP32)
            for t, a in [(idx_t, IDX), (seg_t, SEG), (mt_t, MT), (ea_t, EA),
                         (pool_t, POOL), (id_t, IDENT), (io_t, IOTA),
                         (linw_t, LINW), (linb_t, LINB)]:
                nc.sync.dma_start(t[:], a[:])
            wall_t, qqeb_t, wv_t, we_t, ws_t, hb_t = [], [], [], [], [], []
            for li, L in enumerate(layers):
                F = L["F"]
                wall_t.append(res.tile([F, F + 17], FP16, tag=f"wall{li}", name=f"wall{li}_t"))
                qqeb_t.append(res.tile([128, F + 17], FP32, tag=f"qqeb{li}", name=f"qqeb{li}_t"))
                wv_t.append(res.tile([F, H], FP16, tag=f"wv{li}", name=f"wv{li}_t"))
                we_t.append(res.tile([EDIM, H], FP16, tag=f"we{li}", name=f"we{li}_t"))
                ws_t.append(res.tile([F, H], FP16, tag=f"ws{li}", name=f"ws{li}_t"))
                hb_t.append(res.tile([128, H], FP32, tag=f"hb{li}", name=f"hb{li}_t"))
                for t, a in [(wall_t[li], LW[li]), (qqeb_t[li], LB[li]),
                             (wv_t[li], LV[li]), (we_t[li], LE[li]),
                             (ws_t[li], LS[li]), (hb_t[li], LH[li])]:
                    nc.sync.dma_start(t[:], a[:])

            # own feature-major h (layer input); layer1 = x
            ofm1 = res.tile([F_IN, NLOC], FP16, tag="ofm1")
            nc.sync.dma_start(ofm1[:], XFM[:])
            ofm23 = [res.tile([H, NLOC], FP16, tag="ofm2", name="ofm2_t"),
                     res.tile([H, NLOC], FP16, tag="ofm3", name="ofm3_t")]
            h3loc = res.tile([128, CH * H], FP16, tag="h3loc")

            gshared = [
                dram.tile([NTOT, H], FP16, addr_space="Shared", tag="agA", name="agA_t"),
                dram.tile([NTOT, H], FP16, addr_space="Shared", tag="agB", name="agB_t"),
            ]


            def layer(li):
                L = layers[li]
                F = L["F"]
                Wd = F + 17
                ofm = ofm1 if li == 0 else ofm23[li - 1]

                # ---- node phase: QQE [128, CH*Wd] fp16
                qqe = res.tile([128, CH * Wd], FP16, tag=f"qqe{li}")
                for k in range(CH):
                    pq = psq.tile([128, Wd], FP32, tag="pq")
                    nc.tensor.matmul(pq[:], ofm[:, k * 128:(k + 1) * 128],
                                     wall_t[li][:], start=True, stop=True)
                    nc.vector.tensor_tensor(
                        out=qqe[:, k * Wd:(k + 1) * Wd], in0=pq[:],
                        in1=qqeb_t[li][:, :Wd], op=OP.add)

                # ---- edge phase
                slab = dram.tile([NLOC, H], FP16, tag="slab")
                nblk_done = 0
                sb_open = None
                poutH = None
                poutE = None

                def close_superblock(sb):
                    # poutH [128, F] = aggH ; poutE [128, 17] = aggEA | denom
                    dmx = sbp.tile([128, 1], FP32, tag="dmx")
                    rden = sbp.tile([128, 1], FP32, tag="rden")
                    nc.vector.tensor_scalar_max(dmx[:], poutE[:, 16:17], 1e-20)
                    nc.vector.reciprocal(rden[:], dmx[:])
                    agg = sbp.tile([128, Wd], FP16, tag="agg")
                    nc.scalar.activation(agg[:, 0:F], poutH[:], AF.Copy, scale=rden[:])
                    nc.scalar.activation(agg[:, F:Wd], poutE[:], AF.Copy, scale=rden[:])
                    # transposes
                    pt = psp.tile([128, 256], FP16, tag="pt")
                    nc.tensor.transpose(pt[0:F, 0:128], agg[:, 0:F], id_t[:])
                    nc.tensor.transpose(pt[0:EDIM, 128:256], agg[:, F:F + EDIM], id_t[:])
                    at1 = sbp.tile([F, 128], FP16, tag="at1")
                    nc.scalar.activation(at1[:], pt[0:F, 0:128], AF.Copy)
                    at2 = sbp.tile([EDIM, 128], FP16, tag="at2")
                    nc.scalar.activation(at2[:], pt[0:EDIM, 128:256], AF.Copy)
                    # h' = aggH@wv + aggEA@we + h@ws  (+bias, relu)
                    ph = psp.tile([128, H], FP32, tag="ph")
                    nc.tensor.matmul(ph[:], at1[:], wv_t[li][:], start=True, stop=False)
                    nc.tensor.matmul(ph[:], at2[:], we_t[li][:], start=False, stop=False)
                    nc.tensor.matmul(ph[:], ofm[:, sb * 128:(sb + 1) * 128],
                                     ws_t[li][:], start=False, stop=True)
                    hn = sbp.tile([128, H], FP16, tag="hn")
                    nc.vector.tensor_tensor(out=hn[:], in0=ph[:], in1=hb_t[li][:],
                                            op=OP.add)
                    if li < 2:
                        hr = sbp.tile([128, H], FP16, tag="hr")
                        nc.scalar.activation(hr[:], hn[:], AF.Relu)
                    else:
                        hr = hn
                    if li == 2:
                        nc.vector.tensor_copy(h3loc[:, sb * H:(sb + 1) * H], hr[:])
                    else:
                        # own_fm for next layer + slab for allgather
                        ptn = psp.tile([128, 128], FP16, tag="pt")
                        nc.tensor.transpose(ptn[:], hr[:], id_t[:])
                        nc.scalar.activation(
                            ofm23[li][:, sb * 128:(sb + 1) * 128], ptn[:], AF.Copy)
                        nc.sync.dma_start(slab[sb * 128:(sb + 1) * 128, :], hr[:])

                # chunk loop
                for ch in range(NCHUNK):
                    b0 = ch * GBLK
                    nb = min(GBLK, NBLK - b0)
                    if li == 0:
                        hg = xs.tile([128, GBLK * F_IN], FP16, tag="xs")
                        nc.sync.dma_start(hg[:, :nb * F_IN],
                                          XG[:, b0 * F_IN:(b0 + nb) * F_IN])
                        hgw = F_IN
                        hgt = hg
                    else:
                        hgt = gat.tile([128, GBLK, H], FP16, tag="gat")
                        if os.environ.get("K_SKIP_GATHER"):
                            nc.gpsimd.memset(hgt[:, 0:nb, :], 0.0)
                        else:
                            nc.gpsimd.dma_gather(
                                hgt[:, 0:nb, :], HEXT[li - 1][:],
                                idx_t[:, b0 * 8:(b0 + nb) * 8],
                                num_idxs=nb * KB, num_idxs_reg=nb * KB, elem_size=H,
                                single_packet=False)
                        hgw = H
                    alpha = chk.tile([128, GBLK], FP32, tag="alpha")
                    # per-block alpha
                    for j in range(nb):
                        b = b0 + j
                        ph4 = phase[b]
                        w = wofb[b]
                        kchunk = w // 4
                        pq = psq.tile([128, Wd], FP32, tag="pq")
                        nc.tensor.matmul(
                            pq[:], mt_t[ph4 * 32:ph4 * 32 + 32,
                                        slot[b] * 128:slot[b] * 128 + 128],
                            qqe[ph4 * 32:ph4 * 32 + 32,
                                kchunk * Wd:(kchunk + 1) * Wd],
                            start=True, stop=True, tile_position=(ph4 * 32, 0))
                        prod = blk.tile([128, Wd], FP16, tag="prod")
                        if li == 0:
                            hga = hgt[:, j * F_IN:(j + 1) * F_IN]
                        else:
                            hga = hgt[:, j, :]
                        nc.vector.tensor_tensor(out=prod[:, 0:F], in0=hga,
                                                in1=pq[:, 0:F], op=OP.mult)
                        nc.vector.tensor_tensor(
                            out=prod[:, F:Wd], in0=ea_t[:, b * 17:(b + 1) * 17],
                            in1=pq[:, F:Wd], op=OP.mult)
                        scr = blk.tile([128, Wd], FP16, tag="scr")
                        nc.scalar.activation(scr[:], prod[:], AF.Copy,
                                             accum_out=alpha[:, j:j + 1])
                    exch = chk.tile([128, GBLK], FP32, tag="exch")
                    nc.scalar.activation(exch[:, 0:nb], alpha[:, 0:nb], AF.Exp)
                    # per-block scatter
                    for j in range(nb):
                        b = b0 + j
                        w = wofb[b]
                        ph4 = phase[b]
                        sb = w // 4
                        if sb_open is None or sb != sb_open:
                            if sb_open is not None:
                                close_superblock(sb_open)
                            sb_open = sb
                            poutH = pso.tile([128, F], FP32, tag="poutH")
                            poutE = pso.tile([128, 17], FP32, tag="poutE")
                        sx = blk.tile([128, WIN], FP16, tag="sx")
                        nc.vector.tensor_scalar(
                            out=sx[:], in0=io_t[:], scalar1=seg_t[:, b:b + 1],
                            scalar2=exch[:, j:j + 1], op0=OP.is_equal, op1=OP.mult)
                        first = (b == 0) or (wofb[b - 1] != w)
                        last = (b == NBLK - 1) or (wofb[b + 1] != w)
                        if li == 0:
                            hga = hgt[:, j * F_IN:(j + 1) * F_IN]
                        else:
                            hga = hgt[:, j, :]
                        nc.tensor.matmul(
                            poutH[ph4 * 32:ph4 * 32 + 32, :], sx[:], hga,
                            start=first, stop=last, tile_position=(0, ph4 * 32),
                            skip_group_check=True)
                        nc.tensor.matmul(
                            poutE[ph4 * 32:ph4 * 32 + 32, :], sx[:],
                            ea_t[:, b * 17:(b + 1) * 17],
                            start=first, stop=last, tile_position=(0, ph4 * 32),
                            skip_group_check=True)
                close_superblock(sb_open)
                sb_open = None

                if li < 2:
                    nc.gpsimd.collective_compute(
                        "AllGather", OP.bypass,
                        replica_groups=[list(range(NCORES))],
                        ins=[slab.opt()], outs=[gshared[li].opt()])
                    nc.sync.dma_start(HEXT[li][:], gshared[li][:])

            import os
            nlayers = int(os.environ.get("K_NLAYERS", "3"))
            for _li in range(nlayers):
                layer(_li)
            if nlayers < 3:
                nc.vector.tensor_copy(h3loc[:], h3loc[:]) if False else None
                z = sbp.tile([128, CH * H], FP16, tag="zf")
                nc.gpsimd.memset(z[:], 0.0)
                nc.vector.tensor_copy(h3loc[:], z[:])

            # ---- pooling + classifier
            pp = psp.tile([gper, H], FP32, tag="ph")
            for k in range(CH):
                nc.tensor.matmul(pp[:], pool_t[:, k * gper:(k + 1) * gper],
                                 h3loc[:, k * H:(k + 1) * H],
                                 start=(k == 0), stop=(k == CH - 1))
            pool_s = sbp.tile([gper, H], FP16, tag="pool_s")
            nc.scalar.activation(pool_s[:], pp[:], AF.Copy)
            ppt = psp.tile([H, gper], FP16, tag="pt")
            nc.tensor.transpose(ppt[:], pool_s[:], id_t[0:gper, 0:gper])
            poolT = sbp.tile([H, gper], FP16, tag="poolT")
            nc.scalar.activation(poolT[:], ppt[:], AF.Copy)
            pl = psp.tile([gper, C], FP32, tag="ph")
            nc.tensor.matmul(pl[:], poolT[:], linw_t[:], start=True, stop=True)
            logits = sbp.tile([gper, C], FP32, tag="logits")
            nc.vector.tensor_tensor(out=logits[:], in0=pl[:], in1=linb_t[:],
                                    op=OP.add)
            nc.sync.dma_start(OUT[:], logits[:])

    nc.compile()
    return nc


# --------------------------------------------------------------------- glue --

def _make_in_maps(dims, shared, per_core):
    iota = np.tile(np.arange(WIN, dtype=np.float16), (128, 1))
    ident = np.eye(128, dtype=np.float16)
    ims = []
    for c in range(NCORES):
        pc = per_core[c]
        im = dict(
            idx=pc["idx"], seg=pc["seg"], mt=pc["mt"], ea17=pc["ea17"],
            xg=pc["xg"], xfm=pc["xfm"], pool=pc["pool"],
            ident=ident, iota32=iota,
            lin_w=shared["lin_w"], lin_b=shared["lin_b"],
        )
        for li, L in enumerate(shared["layers"]):
            im[f"wall{li}"] = L["W_all"]
            im[f"qqeb{li}"] = L["qqeb"]
            im[f"wv{li}"] = L["wv"]
            im[f"we{li}"] = L["we"]
            im[f"ws{li}"] = L["ws"]
            im[f"hbias{li}"] = L["hbias"]
        ims.append(im)
    return ims


_CACHE = {}


def _get_program(dims, shared):
    key = (dims["NBLK"], dims["NSLOT"], dims["NLOC"], dims["F_IN"], dims["gper"])
    if key not in _CACHE:
        _CACHE[key] = _build(dims, shared)
    return _CACHE[key]


def kernel(x, edge_index, edge_attr, batch, params, _run_mode="hw",
           _results_hook=None):
    x = np.asarray(x)
    edge_index = np.asarray(edge_index)
    edge_attr = np.asarray(edge_attr)
    batch_np = np.asarray(batch)
    dims, shared, per_core, node_start, cnts = _preprocess(
        x, edge_index, edge_attr, batch_np, params)
    nc = _get_program(dims, shared)
    ims = _make_in_maps(dims, shared, per_core)
    if _run_mode == "sim":
        from concourse.bass_interp import MultiCoreSim
        sim = MultiCoreSim(nc, num_cores=NCORES, trace=False)
        for c, core in sim.cores.items():
            for k, v in ims[c].items():
                core.tensor(k)[:] = v
        sim.simulate(check_with_hw=False)
        outs = [np.asarray(sim.cores[c].tensor("out")) for c in range(NCORES)]
        res = None
    else:
        res = bass_utils.run_bass_kernel_spmd(
            nc, ims, core_ids=list(range(NCORES)),
            trace=(_run_mode == "trace"))
        outs = [res.results[c]["out"] for c in range(NCORES)]
    if _results_hook is not None:
        _results_hook(res)
    return np.concatenate(outs, 0).astype(np.float32)
